# revision 5
# baseline (speedup 1.0000x reference)
"""Clipped shallow PLRNN recurrence on 8 TRN2 NeuronCores (Bass/Tile).

z_{t+1} = A*z_t + (relu(z_t@W2 + h2) - relu(z_t@W2)) @ W1 + h1
x_t     = z_t @ OB + Ob            -> output [bs, nt+1, ns]

Strategy (data-parallel over batch, 32 lanes/core; sequential scan local):
 - state kept transposed zT [65, B] (64 states + constant ones row)
 - relu(y+h2)-relu(y) == |h2| * clamp(sigma*y/|h2|, -1, 0) + relu(h2):
     W2h = W2 * (sign(h2)/|h2|)  (host precompute)    -> mm1 yields yhat
     g   = min(max(yhat, -1), 0)                      (one DVE op, immediates)
     W1h = W1 * |h2|                                  -> mm2 accumulates g@W1h
     h1p = h1 + relu(h2)@W1  folded into the Az matmul bias row
 - A*z + h1p via one matmul with lhsT = [diag(A); h1p] [65,64], rhs = zT
   (ones row of zT supplies the bias)
 - x_t = z_t@OB + Ob via lhsT = zT [65,B], rhs = [OB; Ob] [65,64]
 - per-core batch split into G groups whose chains interleave on the engines
"""

import sys

sys.path.insert(0, "/opt/trn_rl_repo")

import numpy as np

NS = 64      # n_states
NH = 256     # n_hidden
BS = 256     # batch
NCORES = 8
NT = 4096
B = BS // NCORES          # 32 lanes per core

# tunables
G = 2                     # pipelined batch groups per core
U = 64                    # time steps per For_i iteration (DMA chunk)
OBS = 8                   # obs psum slots per bank (batched psum->sbuf copy)

_F32 = None  # set lazily


def _build_program(nt, u, g_groups, nt_run=None, timing_mode=False,
                   staggered_reset=False, obs_batch=False, no_xdma=False,
                   xdma_engine="sync", xsb_bufs=2):
    import concourse.bacc as bacc
    import concourse.mybir as mybir
    import concourse.tile as tile
    from concourse.bass import ds

    f32 = mybir.dt.float32
    bg = B // g_groups
    if nt_run is None:
        nt_run = nt
    assert nt % u == 0 and nt_run % u == 0 and u % OBS == 0 and u % 2 == 0
    if obs_batch:
        return _build_program_obsbatch(
            nt, u, g_groups, nt_run, timing_mode, staggered_reset
        )

    nc = bacc.Bacc("TRN2", target_bir_lowering=False, debug=False)

    z0t = nc.dram_tensor("z0t", [NS + 1, B], f32, kind="ExternalInput")
    w2h = nc.dram_tensor("w2h", [NS, NH], f32, kind="ExternalInput")
    w1h = nc.dram_tensor("w1h", [NH, NS], f32, kind="ExternalInput")
    azm = nc.dram_tensor("azm", [NS + 1, NS], f32, kind="ExternalInput")
    obb = nc.dram_tensor("obb", [NS + 1, NS], f32, kind="ExternalInput")
    x = nc.dram_tensor("x", [B, nt + 1, NS], f32, kind="ExternalOutput")

    Copy = mybir.ActivationFunctionType.Copy
    AOp = mybir.AluOpType

    with tile.TileContext(nc) as tc:
        from contextlib import ExitStack

        with ExitStack() as ctx:
            const = ctx.enter_context(tc.tile_pool(name="const", bufs=1))
            state = ctx.enter_context(tc.tile_pool(name="state", bufs=1))
            gpool = ctx.enter_context(tc.tile_pool(name="gp", bufs=1))
            xsb = ctx.enter_context(tc.tile_pool(name="xsb", bufs=xsb_bufs))
            ypsum = ctx.enter_context(tc.tile_pool(name="yps", bufs=1, space="PSUM"))
            zpsum = ctx.enter_context(tc.tile_pool(name="zps", bufs=1, space="PSUM"))
            xpsum = ctx.enter_context(tc.tile_pool(name="xps", bufs=2, space="PSUM"))

            w2sb = const.tile([NS, NH], f32, tag="w2")
            w1sb = const.tile([128, 2, NS], f32, tag="w1")
            azsb = const.tile([NS + 1, NS], f32, tag="az")
            obsb = const.tile([NS + 1, NS], f32, tag="ob")
            nc.sync.dma_start(out=w2sb, in_=w2h[:, :])
            nc.sync.dma_start(out=w1sb, in_=w1h.rearrange("(c p) m -> p c m", c=2))
            nc.sync.dma_start(out=azsb, in_=azm[:, :])
            nc.sync.dma_start(out=obsb, in_=obb[:, :])

            # per-group ping-pong state tiles [65, bg]
            zs = []
            for g in range(g_groups):
                za = state.tile([NS + 1, bg], f32, tag=f"zA{g}")
                zb = state.tile([NS + 1, bg], f32, tag=f"zB{g}")
                nc.sync.dma_start(out=za, in_=z0t[:, g * bg:(g + 1) * bg])
                nc.vector.memset(zb[NS:NS + 1, :], 1.0)
                zs.append((za, zb))

            # t = 0 observation (per group; engine APs must sit at partition 0)
            for g in range(g_groups):
                px0 = xpsum.tile([bg, 1, NS], f32, tag=f"xp{g}", name=f"px0{g}")
                nc.tensor.matmul(
                    px0[:, 0, :], lhsT=zs[g][0], rhs=obsb, start=True, stop=True
                )
                x0sb = xsb.tile([bg, 1, NS], f32, tag=f"xt{g}", name=f"x0sb{g}")
                nc.scalar.activation(out=x0sb, in_=px0, func=Copy)
                nc.sync.dma_start(
                    out=x[g * bg:(g + 1) * bg, 0:1, :], in_=x0sb
                )

            with tc.For_i(
                0, nt_run, u, hint_engines=(mybir.EngineType.PE,),
                staggered_reset=staggered_reset,
            ) as it:
                xts = [
                    xsb.tile([bg, u, NS], f32, tag=f"xt{g}", name=f"xt{g}")
                    for g in range(g_groups)
                ]
                pxs = [None] * g_groups
                for k in range(u):
                    for g in range(g_groups):
                        za, zb = zs[g]
                        src, dst = (za, zb) if k % 2 == 0 else (zb, za)
                        # mm1: yhat^T chunks [128, 2, bg]
                        py = ypsum.tile([128, 2, bg], f32, tag=f"y{g}")
                        nc.tensor.matmul(
                            py[:, 0, :], lhsT=w2sb[:, 0:128], rhs=src[0:NS, :],
                            start=True, stop=True,
                        )
                        nc.tensor.matmul(
                            py[:, 1, :], lhsT=w2sb[:, 128:256], rhs=src[0:NS, :],
                            start=True, stop=True,
                        )
                        # clamp to [-1, 0]
                        gt = gpool.tile([128, 2, bg], f32, tag=f"g{g}")
                        nc.vector.tensor_scalar(
                            out=gt, in0=py,
                            scalar1=-1.0, scalar2=0.0,
                            op0=AOp.max, op1=AOp.min,
                        )
                        # z_{t+1} = A*z + h1p + g@W1h   (accumulated in psum)
                        pz = zpsum.tile([NS, bg], f32, tag=f"z{g}")
                        nc.tensor.matmul(pz, lhsT=azsb, rhs=src, start=True, stop=False)
                        nc.tensor.matmul(
                            pz, lhsT=w1sb[:, 0, :], rhs=gt[:, 0, :],
                            start=False, stop=False,
                        )
                        nc.tensor.matmul(
                            pz, lhsT=w1sb[:, 1, :], rhs=gt[:, 1, :],
                            start=False, stop=True,
                        )
                        nc.scalar.activation(out=dst[0:NS, :], in_=pz, func=Copy)
                        # observation of z_{t+1}
                        if k % OBS == 0:
                            pxs[g] = xpsum.tile(
                                [bg, OBS, NS], f32, tag=f"xp{g}", name=f"px{g}"
                            )
                        nc.tensor.matmul(
                            pxs[g][:, k % OBS, :], lhsT=dst, rhs=obsb,
                            start=True, stop=True,
                        )
                        if k % OBS == OBS - 1:
                            nc.scalar.activation(
                                out=xts[g][:, k - (OBS - 1):k + 1, :],
                                in_=pxs[g], func=Copy,
                            )
                for g in range(g_groups):
                    if no_xdma:
                        continue
                    dest_t = ds(1, u) if timing_mode else ds(it + 1, u)
                    eng = nc.gpsimd if xdma_engine == "gpsimd" else nc.sync
                    eng.dma_start(
                        out=x[g * bg:(g + 1) * bg, dest_t, :], in_=xts[g]
                    )

    nc.compile()
    return nc


def _build_program_obsbatch(nt, u, g_groups, nt_run, timing_mode,
                            staggered_reset):
    """Variant: z history kept in an 8-slot rotating buffer per group;
    one [65,8*bg]x[65,64] obs matmul per 8 steps replaces 8 small ones."""
    import concourse.bacc as bacc
    import concourse.mybir as mybir
    import concourse.tile as tile
    from concourse.bass import ds
    from contextlib import ExitStack

    f32 = mybir.dt.float32
    bg = B // g_groups
    S = OBS  # history slots
    assert u % S == 0

    nc = bacc.Bacc("TRN2", target_bir_lowering=False, debug=False)

    z0t = nc.dram_tensor("z0t", [NS + 1, B], f32, kind="ExternalInput")
    w2h = nc.dram_tensor("w2h", [NS, NH], f32, kind="ExternalInput")
    w1h = nc.dram_tensor("w1h", [NH, NS], f32, kind="ExternalInput")
    azm = nc.dram_tensor("azm", [NS + 1, NS], f32, kind="ExternalInput")
    obb = nc.dram_tensor("obb", [NS + 1, NS], f32, kind="ExternalInput")
    x = nc.dram_tensor("x", [B, nt + 1, NS], f32, kind="ExternalOutput")

    Copy = mybir.ActivationFunctionType.Copy
    AOp = mybir.AluOpType

    with tile.TileContext(nc) as tc:
        with ExitStack() as ctx:
            const = ctx.enter_context(tc.tile_pool(name="const", bufs=1))
            state = ctx.enter_context(tc.tile_pool(name="state", bufs=1))
            gpool = ctx.enter_context(tc.tile_pool(name="gp", bufs=1))
            xsb = ctx.enter_context(tc.tile_pool(name="xsb", bufs=xsb_bufs))
            ypsum = ctx.enter_context(tc.tile_pool(name="yps", bufs=1, space="PSUM"))
            zpsum = ctx.enter_context(tc.tile_pool(name="zps", bufs=1, space="PSUM"))
            xpsum = ctx.enter_context(tc.tile_pool(name="xps", bufs=2, space="PSUM"))

            w2sb = const.tile([NS, NH], f32, tag="w2")
            w1sb = const.tile([128, 2, NS], f32, tag="w1")
            azsb = const.tile([NS + 1, NS], f32, tag="az")
            obsb = const.tile([NS + 1, NS], f32, tag="ob")
            nc.sync.dma_start(out=w2sb, in_=w2h[:, :])
            nc.sync.dma_start(out=w1sb, in_=w1h.rearrange("(c p) m -> p c m", c=2))
            nc.sync.dma_start(out=azsb, in_=azm[:, :])
            nc.sync.dma_start(out=obsb, in_=obb[:, :])

            # per-group rotating z history [65, S, bg]; slot j holds z_{t}
            # with t % S == j. ones row constant across slots.
            zh = []
            for g in range(g_groups):
                zhg = state.tile([NS + 1, S, bg], f32, tag=f"zh{g}", name=f"zh{g}")
                nc.vector.memset(zhg[NS:NS + 1, :, :], 1.0)
                # z0 lives in slot S-1 (step k reads slot (k-1) % S)
                nc.sync.dma_start(
                    out=zhg[0:NS, S - 1, :], in_=z0t[0:NS, g * bg:(g + 1) * bg]
                )
                zh.append(zhg)

            # t = 0 observation from slot S-1
            for g in range(g_groups):
                px0 = xpsum.tile([bg, NS], f32, tag=f"xp{g}", name=f"px0{g}")
                nc.tensor.matmul(
                    px0, lhsT=zh[g][:, S - 1, :], rhs=obsb, start=True, stop=True
                )
                x0sb = xsb.tile([bg, NS], f32, tag=f"x0sb{g}", name=f"x0sb{g}")
                nc.scalar.activation(out=x0sb, in_=px0, func=Copy)
                nc.sync.dma_start(
                    out=x[g * bg:(g + 1) * bg, 0, :], in_=x0sb
                )

            with tc.For_i(
                0, nt_run, u, hint_engines=(mybir.EngineType.PE,),
                staggered_reset=staggered_reset,
            ) as it:
                xst = [
                    xsb.tile([S * bg, u // S, NS], f32, tag=f"xt{g}", name=f"xst{g}")
                    for g in range(g_groups)
                ]
                for k in range(u):
                    for g in range(g_groups):
                        zhg = zh[g]
                        src = zhg[:, (k - 1) % S, :]
                        dst = zhg[:, k % S, :]
                        py = ypsum.tile([128, 2, bg], f32, tag=f"y{g}")
                        nc.tensor.matmul(
                            py[:, 0, :], lhsT=w2sb[:, 0:128], rhs=src[0:NS, :],
                            start=True, stop=True,
                        )
                        nc.tensor.matmul(
                            py[:, 1, :], lhsT=w2sb[:, 128:256], rhs=src[0:NS, :],
                            start=True, stop=True,
                        )
                        gt = gpool.tile([128, 2, bg], f32, tag=f"g{g}")
                        nc.vector.tensor_scalar(
                            out=gt, in0=py,
                            scalar1=-1.0, scalar2=0.0,
                            op0=AOp.max, op1=AOp.min,
                        )
                        pz = zpsum.tile([NS, bg], f32, tag=f"z{g}")
                        nc.tensor.matmul(pz, lhsT=azsb, rhs=src, start=True, stop=False)
                        nc.tensor.matmul(
                            pz, lhsT=w1sb[:, 0, :], rhs=gt[:, 0, :],
                            start=False, stop=False,
                        )
                        nc.tensor.matmul(
                            pz, lhsT=w1sb[:, 1, :], rhs=gt[:, 1, :],
                            start=False, stop=True,
                        )
                        nc.scalar.activation(out=dst[0:NS, :], in_=pz, func=Copy)
                        if k % S == S - 1:
                            # batched obs of slots 0..S-1 (= z_{t+1} for the
                            # last S steps, slot-major == t ascending)
                            px = xpsum.tile(
                                [S * bg, NS], f32, tag=f"xp{g}", name=f"px{g}"
                            )
                            nc.tensor.matmul(
                                px, lhsT=zhg, rhs=obsb,
                                start=True, stop=True,
                            )
                            nc.scalar.activation(
                                out=xst[g][:, k // S, :], in_=px, func=Copy
                            )
                for g in range(g_groups):
                    dest_t = ds(1, u) if timing_mode else ds(it + 1, u)
                    dest = x[g * bg:(g + 1) * bg, dest_t, :].rearrange(
                        "b (c s) n -> s b c n", s=S
                    )
                    nc.sync.dma_start(out=dest, in_=xst[g])

    nc.compile()
    return nc


def _build_program_unrolled(nt, g_groups=2, dma_chunk=64, obs_chunk=8,
                            timing_mode=False, nt_steps=None, no_obs=False):
    """Fully unrolled time loop: every DMA destination is static, avoiding the
    ~50MB/s dynamic-DMA ucode path; no For_i back-edge barriers."""
    import concourse.bacc as bacc
    import concourse.mybir as mybir
    import concourse.tile as tile
    from contextlib import ExitStack

    f32 = mybir.dt.float32
    bg = B // g_groups
    if nt_steps is None:
        nt_steps = nt
    assert nt % dma_chunk == 0 and dma_chunk % obs_chunk == 0

    nc = bacc.Bacc("TRN2", target_bir_lowering=False, debug=False)

    z0t = nc.dram_tensor("z0t", [NS + 1, B], f32, kind="ExternalInput")
    w2h = nc.dram_tensor("w2h", [NS, NH], f32, kind="ExternalInput")
    w1h = nc.dram_tensor("w1h", [NH, NS], f32, kind="ExternalInput")
    azm = nc.dram_tensor("azm", [NS + 1, NS], f32, kind="ExternalInput")
    obb = nc.dram_tensor("obb", [NS + 1, NS], f32, kind="ExternalInput")
    x_t_size = (dma_chunk + 1) if timing_mode else (nt + 1)
    x = nc.dram_tensor("x", [B, x_t_size, NS], f32, kind="ExternalOutput")

    Copy = mybir.ActivationFunctionType.Copy
    AOp = mybir.AluOpType

    with tile.TileContext(nc) as tc:
        with ExitStack() as ctx:
            const = ctx.enter_context(tc.tile_pool(name="const", bufs=1))
            state = ctx.enter_context(tc.tile_pool(name="state", bufs=1))
            gpool = ctx.enter_context(tc.tile_pool(name="gp", bufs=1))
            xsb = ctx.enter_context(tc.tile_pool(name="xsb", bufs=2))
            ypsum = ctx.enter_context(tc.tile_pool(name="yps", bufs=1, space="PSUM"))
            zpsum = ctx.enter_context(tc.tile_pool(name="zps", bufs=1, space="PSUM"))
            xpsum = ctx.enter_context(tc.tile_pool(name="xps", bufs=2, space="PSUM"))

            w2sb = const.tile([NS, NH], f32, tag="w2")
            w1sb = const.tile([128, 2, NS], f32, tag="w1")
            azsb = const.tile([NS + 1, NS], f32, tag="az")
            obsb = const.tile([NS + 1, NS], f32, tag="ob")
            nc.sync.dma_start(out=w2sb, in_=w2h[:, :])
            nc.sync.dma_start(out=w1sb, in_=w1h.rearrange("(c p) m -> p c m", c=2))
            nc.sync.dma_start(out=azsb, in_=azm[:, :])
            nc.sync.dma_start(out=obsb, in_=obb[:, :])

            zs = []
            for g in range(g_groups):
                za = state.tile([NS + 1, bg], f32, tag=f"zA{g}", name=f"zA{g}")
                zb = state.tile([NS + 1, bg], f32, tag=f"zB{g}", name=f"zB{g}")
                nc.sync.dma_start(out=za, in_=z0t[:, g * bg:(g + 1) * bg])
                nc.vector.memset(zb[NS:NS + 1, :], 1.0)
                zs.append((za, zb))

            for g in range(g_groups):
                px0 = xpsum.tile([bg, 1, NS], f32, tag=f"xp{g}", name=f"px0{g}")
                nc.tensor.matmul(
                    px0[:, 0, :], lhsT=zs[g][0], rhs=obsb, start=True, stop=True
                )
                x0sb = xsb.tile([bg, 1, NS], f32, tag=f"x0{g}", name=f"x0sb{g}")
                nc.scalar.activation(out=x0sb, in_=px0, func=Copy)
                nc.sync.dma_start(out=x[g * bg:(g + 1) * bg, 0:1, :], in_=x0sb)

            xts = [None] * g_groups
            pxs = [None] * g_groups
            for t in range(nt_steps):
                kd = t % dma_chunk
                for g in range(g_groups):
                    za, zb = zs[g]
                    src, dst = (za, zb) if t % 2 == 0 else (zb, za)
                    if kd == 0:
                        xts[g] = xsb.tile(
                            [bg, dma_chunk, NS], f32, tag=f"xt{g}", name=f"xt{g}"
                        )
                    py = ypsum.tile([128, 2, bg], f32, tag=f"y{g}", name=f"py{g}")
                    nc.tensor.matmul(
                        py[:, 0, :], lhsT=w2sb[:, 0:128], rhs=src[0:NS, :],
                        start=True, stop=True,
                    )
                    nc.tensor.matmul(
                        py[:, 1, :], lhsT=w2sb[:, 128:256], rhs=src[0:NS, :],
                        start=True, stop=True,
                    )
                    gt = gpool.tile([128, 2, bg], f32, tag=f"g{g}", name=f"gt{g}")
                    nc.vector.tensor_scalar(
                        out=gt, in0=py, scalar1=-1.0, scalar2=0.0,
                        op0=AOp.max, op1=AOp.min,
                    )
                    pz = zpsum.tile([NS, bg], f32, tag=f"z{g}", name=f"pz{g}")
                    nc.tensor.matmul(pz, lhsT=azsb, rhs=src, start=True, stop=False)
                    nc.tensor.matmul(
                        pz, lhsT=w1sb[:, 0, :], rhs=gt[:, 0, :],
                        start=False, stop=False,
                    )
                    nc.tensor.matmul(
                        pz, lhsT=w1sb[:, 1, :], rhs=gt[:, 1, :],
                        start=False, stop=True,
                    )
                    nc.scalar.activation(out=dst[0:NS, :], in_=pz, func=Copy)
                    if no_obs:
                        continue
                    if t % obs_chunk == 0:
                        pxs[g] = xpsum.tile(
                            [bg, obs_chunk, NS], f32, tag=f"xp{g}", name=f"px{g}"
                        )
                    nc.tensor.matmul(
                        pxs[g][:, t % obs_chunk, :], lhsT=dst, rhs=obsb,
                        start=True, stop=True,
                    )
                    if t % obs_chunk == obs_chunk - 1:
                        nc.scalar.activation(
                            out=xts[g][:, kd - (obs_chunk - 1):kd + 1, :],
                            in_=pxs[g], func=Copy,
                        )
                    if kd == dma_chunk - 1:
                        t0 = 0 if timing_mode else t - (dma_chunk - 1)
                        nc.sync.dma_start(
                            out=x[g * bg:(g + 1) * bg, t0 + 1:t0 + 1 + dma_chunk, :],
                            in_=xts[g],
                        )

    nc.compile()
    return nc


U2 = 128   # v2 time steps per For_i iteration
S2 = 4     # v2 obs history slots (S2*B = 128 partitions for obs matmul)


def _build_program_v2(nt, u, nt_run=None, timing_mode=False):
    """v2: G=1 (all 32 lanes in one chain), bf16 for the mm1 / g@W1h / obs
    matmul paths (both operands bf16), fp32 state carry for the A*z matmul.
    Per step:
      py  = w2b^T zh[prev]          (2 bf16 matmuls, psum fp32)
      g   = clamp(py, -1, 0)        (1 DVE op, bf16 out)
      pz  = azm^T z32 + w1b^T g     (1 fp32 + 2 bf16 matmuls, one psum group)
      zh[slot] = bf16(pz)  [DVE]    z32 = fp32(pz)  [Act]
    obs: every S2 steps one [65,128]x[65,64] bf16 matmul over the zh history,
    one Act copy to the x staging tile; DMA per u-chunk."""
    import concourse.bacc as bacc
    import concourse.mybir as mybir
    import concourse.tile as tile
    from concourse.bass import ds
    from contextlib import ExitStack

    f32 = mybir.dt.float32
    bf16 = mybir.dt.bfloat16
    if nt_run is None:
        nt_run = nt
    S = S2
    assert nt % u == 0 and nt_run % u == 0 and u % S == 0

    nc = bacc.Bacc("TRN2", target_bir_lowering=False, debug=False)

    bq = BS // NCORES  # 32 lanes
    z0t = nc.dram_tensor("z0t", [NS + 1, bq], f32, kind="ExternalInput")
    w2b_d = nc.dram_tensor("w2b", [NS, NH], bf16, kind="ExternalInput")
    w1b_d = nc.dram_tensor("w1b", [NH, NS], bf16, kind="ExternalInput")
    azm_d = nc.dram_tensor("azm", [NS + 1, NS], f32, kind="ExternalInput")
    obb_d = nc.dram_tensor("obb", [NS + 1, NS], bf16, kind="ExternalInput")
    x_t_size = (u + 1) if timing_mode else (nt + 1)
    x = nc.dram_tensor("x", [bq, x_t_size, NS], f32, kind="ExternalOutput")

    Copy = mybir.ActivationFunctionType.Copy
    AOp = mybir.AluOpType

    with tile.TileContext(nc) as tc:
        with ExitStack() as ctx:
            const = ctx.enter_context(tc.tile_pool(name="const", bufs=1))
            state = ctx.enter_context(tc.tile_pool(name="state", bufs=1))
            gpool = ctx.enter_context(tc.tile_pool(name="gp", bufs=1))
            xsb = ctx.enter_context(tc.tile_pool(name="xsb", bufs=2))
            ypsum = ctx.enter_context(tc.tile_pool(name="yps", bufs=2, space="PSUM"))
            zpsum = ctx.enter_context(tc.tile_pool(name="zps", bufs=2, space="PSUM"))
            xpsum = ctx.enter_context(tc.tile_pool(name="xps", bufs=2, space="PSUM"))

            w2sb = const.tile([NS, 2, 128], bf16, tag="w2")
            w1sb = const.tile([128, 2, NS], bf16, tag="w1")
            azsb = const.tile([NS + 1, NS], f32, tag="az")
            obsb = const.tile([NS + 1, NS], bf16, tag="ob")
            nc.sync.dma_start(out=w2sb, in_=w2b_d.rearrange("k (c m) -> k c m", c=2))
            nc.sync.dma_start(out=w1sb, in_=w1b_d.rearrange("(c p) m -> p c m", c=2))
            nc.sync.dma_start(out=azsb, in_=azm_d[:, :])
            nc.sync.dma_start(out=obsb, in_=obb_d[:, :])

            # fp32 state (for the A*z matmul); ones row comes in via z0t
            z32 = state.tile([NS + 1, bq], f32, tag="z32", name="z32")
            nc.sync.dma_start(out=z32, in_=z0t[:, :])
            # bf16 state history [65, S, bq]; slot S-1 starts as bf16(z0)
            zh = state.tile([NS + 1, S, bq], bf16, tag="zh", name="zh")
            nc.vector.memset(zh[NS:NS + 1, :, :], 1.0)
            nc.vector.tensor_scalar(out=zh[0:NS, S - 1, :], in0=z32[0:NS, :],
                                    scalar1=0.0, scalar2=0.0,
                                    op0=AOp.add, op1=AOp.add)

            # t = 0 observation from slot S-1
            px0 = xpsum.tile([bq, NS], f32, tag="xp0", name="px0")
            nc.tensor.matmul(px0, lhsT=zh[:, S - 1, :], rhs=obsb,
                             start=True, stop=True)
            x0sb = xsb.tile([bq, NS], f32, tag="x0", name="x0sb")
            nc.scalar.activation(out=x0sb, in_=px0, func=Copy)
            nc.sync.dma_start(out=x[:, 0, :], in_=x0sb)

            with tc.For_i(
                0, nt_run, u, hint_engines=(mybir.EngineType.PE,),
            ) as it:
                xst = xsb.tile([S * bq, u // S, NS], f32, tag="xt", name="xst")
                for k in range(u):
                    slot, prev = k % S, (k - 1) % S
                    py = ypsum.tile([128, 2, bq], f32, tag="y")
                    nc.tensor.matmul(py[:, 0, :], lhsT=w2sb[:, 0, :],
                                     rhs=zh[0:NS, prev, :],
                                     start=True, stop=True)
                    nc.tensor.matmul(py[:, 1, :], lhsT=w2sb[:, 1, :],
                                     rhs=zh[0:NS, prev, :],
                                     start=True, stop=True)
                    gt = gpool.tile([128, 2, bq], bf16, tag="g")
                    nc.vector.tensor_scalar(out=gt, in0=py,
                                            scalar1=-1.0, scalar2=0.0,
                                            op0=AOp.max, op1=AOp.min)
                    pz = zpsum.tile([NS, bq], f32, tag="z")
                    nc.tensor.matmul(pz, lhsT=azsb, rhs=z32,
                                     start=True, stop=False)
                    nc.tensor.matmul(pz, lhsT=w1sb[:, 0, :], rhs=gt[:, 0, :],
                                     start=False, stop=False)
                    nc.tensor.matmul(pz, lhsT=w1sb[:, 1, :], rhs=gt[:, 1, :],
                                     start=False, stop=True)
                    # dual state copy: bf16 (chain) on DVE, fp32 on Act
                    nc.vector.tensor_scalar(out=zh[0:NS, slot, :], in0=pz,
                                            scalar1=0.0, scalar2=0.0,
                                            op0=AOp.add, op1=AOp.add)
                    nc.scalar.activation(out=z32[0:NS, :], in_=pz, func=Copy)
                    if k % S == S - 1:
                        px = xpsum.tile([S * bq, NS], f32, tag="xp", name="px")
                        nc.tensor.matmul(px, lhsT=zh, rhs=obsb,
                                         start=True, stop=True)
                        nc.scalar.activation(out=xst[:, k // S, :], in_=px,
                                             func=Copy)
                for s in range(S):
                    dest_t = (ds(1 + s, u // S, S) if timing_mode
                              else ds(it + 1 + s, u // S, S))
                    nc.sync.dma_start(out=x[:, dest_t, :],
                                      in_=xst[s * bq:(s + 1) * bq, :, :])

    nc.compile()
    return nc


_prog_cache = {}


def _get_program(nt, u, g_groups):
    key = (nt, u, g_groups)
    if key not in _prog_cache:
        _prog_cache[key] = _build_program(nt, u, g_groups)
    return _prog_cache[key]


def make_runner(nc, n_cores=NCORES):
    """Multi-core PJRT runner (mirrors bass2jax.run_bass_via_pjrt), with a
    unique jit body name per program: the neuron NEFF disk cache keys on the
    module file_prefix, which ignores the embedded BIR — identical I/O
    signatures would otherwise collide across different programs."""
    import uuid

    import jax
    import concourse.mybir as mybir
    from concourse import bass2jax
    from concourse.bass2jax import _bass_exec_p, partition_id_tensor
    from jax.sharding import Mesh, PartitionSpec
    from jax.experimental.shard_map import shard_map

    bass2jax.install_neuronx_cc_hook()
    partition_name = nc.partition_id_tensor.name if nc.partition_id_tensor else None
    in_names, out_names, out_avals, zero_outs = [], [], [], []
    for alloc in nc.m.functions[0].allocations:
        if not isinstance(alloc, mybir.MemoryLocationSet):
            continue
        name = alloc.memorylocations[0].name
        if alloc.kind == "ExternalInput":
            if name != partition_name:
                in_names.append(name)
        elif alloc.kind == "ExternalOutput":
            shape = tuple(alloc.tensor_shape)
            dtype = mybir.dt.np(alloc.dtype)
            out_names.append(name)
            out_avals.append(jax.core.ShapedArray(shape, dtype))
            zero_outs.append(np.zeros(shape, dtype))
    n_params = len(in_names)
    n_outs = len(out_avals)
    in_names_all = in_names + out_names + ([partition_name] if partition_name else [])
    donate = tuple(range(n_params, n_params + n_outs))

    def _body(*args):
        operands = list(args)
        if partition_name is not None:
            operands.append(partition_id_tensor())
        outs = _bass_exec_p.bind(
            *operands,
            out_avals=tuple(out_avals),
            in_names=tuple(in_names_all),
            out_names=tuple(out_names),
            lowering_input_output_aliases=(),
            sim_require_finite=True,
            sim_require_nnan=True,
            nc=nc,
        )
        return tuple(outs)

    _body.__name__ = f"body_{uuid.uuid4().hex[:12]}"

    devices = jax.devices()[:n_cores]
    assert len(devices) == n_cores
    mesh = Mesh(np.asarray(devices), ("core",))
    sharded = jax.jit(
        shard_map(
            _body, mesh=mesh,
            in_specs=(PartitionSpec("core"),) * (n_params + n_outs),
            out_specs=(PartitionSpec("core"),) * n_outs,
            check_rep=False,
        ),
        donate_argnums=donate,
        keep_unused=True,
    )

    def run(in_maps):
        import time as _time

        per_core = [[np.asarray(m[n]) for n in in_names] for m in in_maps]
        concat_in = [
            np.concatenate([per_core[c][i] for c in range(n_cores)], 0)
            for i in range(n_params)
        ]
        concat_zeros = [
            np.zeros((n_cores * z.shape[0], *z.shape[1:]), z.dtype)
            for z in zero_outs
        ]
        t0 = _time.time()
        out = sharded(*concat_in, *concat_zeros)
        out = [np.asarray(o) for o in out]
        dt = _time.time() - t0
        res = [
            {
                n: out[i].reshape(n_cores, *out_avals[i].shape)[c]
                for i, n in enumerate(out_names)
            }
            for c in range(n_cores)
        ]
        return res, dt

    return run


def _host_precompute(z0, A, W1, W2, h1, h2, OB, Ob):
    h2d = h2.astype(np.float64)
    sigma = np.where(h2d >= 0, 1.0, -1.0)
    absh = np.maximum(np.abs(h2d), 1e-30)
    w2h = (W2.astype(np.float64) * (sigma / absh)[None, :]).astype(np.float32)
    w1h = (W1.astype(np.float64) * absh[:, None]).astype(np.float32)
    h1p = (h1.astype(np.float64) + np.maximum(h2d, 0) @ W1.astype(np.float64)).astype(
        np.float32
    )
    azm = np.zeros([NS + 1, NS], np.float32)
    azm[np.arange(NS), np.arange(NS)] = A.astype(np.float32)
    azm[NS] = h1p
    obb = np.concatenate(
        [OB.astype(np.float32), Ob.astype(np.float32)[None, :]], axis=0
    )
    return w2h, w1h, azm, obb


def kernel(**inputs):
    import ml_dtypes

    z0 = np.asarray(inputs["z0"], np.float32)
    A = np.asarray(inputs["A"], np.float32)
    W1 = np.asarray(inputs["W1"], np.float32)
    W2 = np.asarray(inputs["W2"], np.float32)
    h1 = np.asarray(inputs["h1"], np.float32)
    h2 = np.asarray(inputs["h2"], np.float32)
    OB = np.asarray(inputs["OB"], np.float32)
    Ob = np.asarray(inputs["Ob"], np.float32)
    nt = int(inputs["nt"])
    assert nt == NT and z0.shape == (BS, NS)

    w2h, w1h, azm, obb = _host_precompute(z0, A, W1, W2, h1, h2, OB, Ob)
    bf16 = ml_dtypes.bfloat16
    w2b = w2h.astype(bf16)
    w1b = w1h.astype(bf16)
    obb16 = obb.astype(bf16)

    in_maps = []
    for c in range(NCORES):
        zslice = z0[c * B:(c + 1) * B]  # [B, NS]
        z0t = np.concatenate(
            [zslice.T.copy(), np.ones([1, B], np.float32)], axis=0
        )  # [NS+1, B]
        in_maps.append(
            {"z0t": z0t, "w2b": w2b, "w1b": w1b, "azm": azm, "obb": obb16}
        )

    key = ("v2", nt, U2)
    if key not in _prog_cache:
        _prog_cache[key] = _build_program_v2(nt, U2)
    nc = _prog_cache[key]
    run = make_runner(nc)
    global _last_in_maps, _last_runner
    _last_in_maps, _last_runner = in_maps, run
    results, _ = run(in_maps)
    xs = [results[c]["x"] for c in range(NCORES)]
    return np.concatenate(xs, axis=0).astype(np.float32)


_last_in_maps = None
_last_runner = None

